# revision 10
# baseline (speedup 1.0000x reference)
"""Trainium2 Bass kernel for nn_Dense_BinaryLayer (binary-weight dense layer).

out = x @ Wb + b, where Wb = binarize(W) in {-1, +1}.

Strategy: data-parallel over the 8 NeuronCores — each core handles 2048 rows
of x with replicated W and b; no collectives.  Host-side prep is layout-only
byte movement: x is sharded and re-blocked so every DMA lands as 128
contiguous >=2KB descriptors (HWDGE descriptor generation is ~5ns/desc and
serial, so descriptor count — not just bytes — gates the pipe), W is passed
as its upper 2 bytes per f32 (== truncated bf16; only the sign/threshold
bit matters for binarize) halving W traffic, and b is passed as [128, 8] so
bias lands on partitions.

Per core (v3 — W-stationary, x-moving, transposed output):
  - One HWDGE queue (nc.sync) carries all bulk traffic in exact need-order:
    (W k-slab, x rc0 k-slab) pairs, then x rc1 halves, rc2, rc3, then
    stores.  No SWDGE, no cast DMAs (xT is declared float32r, bit-identical
    to f32 — the PE rounds internally).
  - W slab kt binarizes on DVE the moment it lands, one fused op:
    wb = (W > 2^-24) - 0.5 in {-0.5,+0.5} f32r; the missing x2 is folded
    into the eviction scale.
  - Matmul: stationary = wb block [128k x 128j], moving = x [128k x 512r],
    PSUM bank = [128j x 512r]; 8 banks = one 512-row block across all 1024
    output columns.  k-outer for row-blocks 0-1 (consumes W slabs in
    arrival order), j-outer for 2-3 (staggers bank completion so eviction
    overlaps the next block).
  - 6 dummy matmuls on a zeroed tile warm the PE HAM clock gate
    (1.2 -> 2.4 GHz) during the DMA ramp, ending just as real data lands.
  - Evictions alternate ACT/DVE: out_bf16 = 2*psum + b[j] (per-partition
    bias).  bf16 output halves store traffic; ~3e-3 max rel err (budget
    2e-2).  Host: out = concat(outT.T per core).astype(f32).
"""
import sys

sys.path.insert(0, "/opt/trn_rl_repo")

import numpy as np

N_TOTAL = 16384
D_IN = 1024
D_OUT = 1024
N_CORES = 8
ROWS = N_TOTAL // N_CORES      # 2048 rows per core
P = 128
KT = D_IN // P                 # 8 k-tiles
JT = D_OUT // P                # 8 j-tiles (psum banks)
RC = 4                         # row chunks per core
RCHUNK = ROWS // RC            # 512 rows = one full PSUM bank of f32
BIN_THRESH = 2.0 ** -24

_cached = {}


def _build():
    import concourse.tile as tile
    from concourse import bacc, mybir

    f32 = mybir.dt.float32
    f32r = mybir.dt.float32r
    bf16 = mybir.dt.bfloat16
    TS = mybir.AluOpType
    ACT = mybir.ActivationFunctionType

    nc = bacc.Bacc()
    # x, blocked per row-chunk: xB[rc, p, kt, r] = x[rc*512+r, kt*128+p]
    x_d = nc.declare_dram_parameter("xB", [RC, P, KT, RCHUNK], f32r,
                                    isOutput=False)
    # W as truncated bf16, blocked: wB[p, kt, j] = bf16(W[kt*128+p, j])
    w_d = nc.declare_dram_parameter("wB", [P, KT, D_OUT], bf16, isOutput=False)
    b_d = nc.declare_dram_parameter("bT", [P, JT], f32, isOutput=False)
    o_d = nc.declare_dram_parameter("outT", [D_OUT, ROWS], bf16, isOutput=True)

    with tile.TileContext(nc) as tc:
        with (
            tc.tile_pool(name="const", bufs=1) as const,
            tc.tile_pool(name="wpool", bufs=1) as wpool,
            tc.tile_pool(name="xp", bufs=4) as xp,
            tc.tile_pool(name="outp", bufs=32) as outp,
            tc.tile_pool(name="pso", bufs=8, space="PSUM") as pso,
        ):
            # --- persistent SBUF tiles
            warm = const.tile([P, RCHUNK], f32, tag="warm")
            bias = const.tile([P, JT], f32, tag="bias")
            w_raw = wpool.tile([P, KT, D_OUT], bf16, tag="wraw")
            wb = wpool.tile([P, KT, D_OUT], f32r, tag="wb")
            x_t = [xp.tile([P, KT, RCHUNK], f32r, tag="x", name=f"x{rc}")
                   for rc in range(RC)]
            ps = [pso.tile([P, RCHUNK], f32, tag="ps", name=f"ps{j}")
                  for j in range(JT)]

            # --- HAM warm-up: PE busy from ~0.3us after engine start so the
            # clock gate flips to 8/8 as real data lands (bank 7 is reset by
            # the first real start=True matmul)
            nc.vector.memset(warm[:], 0.0)
            warm_r = warm[:].bitcast(f32r)
            for _ in range(7):
                nc.tensor.matmul(ps[7][:], warm_r[:, 0:P], warm_r[:],
                                 start=True, stop=True)

            # --- bias on the second HWDGE queue (tiny, done early)
            nc.scalar.dma_start(bias[:], b_d[:])

            # --- sync HWDGE queue, strict need-order; binarize each W slab
            # on DVE the moment it lands
            for kt in range(KT):
                nc.sync.dma_start(w_raw[:, kt, :], w_d[:, kt, :])
                nc.sync.dma_start(x_t[0][:, kt, :], x_d[0, :, kt, :])
                nc.vector.tensor_scalar(
                    wb[:, kt, :], w_raw[:, kt, :], BIN_THRESH, 0.5,
                    TS.is_gt, TS.subtract,
                )
            nc.sync.dma_start(x_t[1][:, 0:4, :], x_d[1, :, 0:4, :])
            nc.sync.dma_start(x_t[1][:, 4:8, :], x_d[1, :, 4:8, :])
            nc.sync.dma_start(x_t[2][:], x_d[2])
            nc.sync.dma_start(x_t[3][:], x_d[3])

            def evict_store(rc, jt):
                # out_bf16 = 2*psum + b[j]; alternate engines so bank drain
                # keeps pace with the matmul stream
                o_sb = outp.tile([P, RCHUNK], bf16, tag="o", name=f"o{rc}_{jt}")
                bcol = bias[:, jt:jt + 1]
                if jt % 2 == 0:
                    nc.scalar.activation(o_sb[:], ps[jt][:], ACT.Identity,
                                         bias=bcol, scale=2.0)
                else:
                    nc.vector.tensor_scalar(o_sb[:], ps[jt][:], 2.0, bcol,
                                            TS.mult, TS.add)
                nc.sync.dma_start(
                    o_d[jt * P:(jt + 1) * P, rc * RCHUNK:(rc + 1) * RCHUNK],
                    o_sb[:])

            def mm(rc, jt, kt):
                nc.tensor.matmul(
                    ps[jt][:],
                    wb[:, kt, jt * P:(jt + 1) * P],
                    x_t[rc][:, kt, :],
                    start=(kt == 0), stop=(kt == KT - 1),
                )

            # row-blocks 0-1: k-outer (follows W/x arrival order)
            for rc in range(2):
                for kt in range(KT):
                    for jt in range(JT):
                        mm(rc, jt, kt)
                for jt in range(JT):
                    evict_store(rc, jt)
            # row-blocks 2-3: j-outer (staggered bank completion); rc3 ends
            # on an ACT-evicted group for a slightly shorter tail
            for rc, jts in ((2, range(JT)), (3, (0, 1, 2, 3, 4, 5, 7, 6))):
                for jt in jts:
                    for kt in range(KT):
                        mm(rc, jt, kt)
                    evict_store(rc, jt)

    nc.compile()
    nc.finalize()
    return nc


def _make_in_maps(x, W, b):
    import ml_dtypes

    x = np.asarray(x, dtype=np.float32)
    W = np.ascontiguousarray(np.asarray(W, dtype=np.float32))
    # upper 2 bytes of each little-endian f32 == truncated bf16
    w_hi = W.view(np.uint16).reshape(D_IN, D_OUT, 2)[:, :, 1]
    wB = np.ascontiguousarray(
        w_hi.reshape(KT, P, D_OUT).transpose(1, 0, 2)).view(ml_dtypes.bfloat16)
    bT = np.ascontiguousarray(
        np.asarray(b, dtype=np.float32).reshape(JT, P).T)

    def x_blocked(c):
        xT = x[c * ROWS:(c + 1) * ROWS].T          # [D_IN, ROWS]
        xB = xT.reshape(KT, P, RC, RCHUNK).transpose(2, 1, 0, 3)
        return np.ascontiguousarray(xB)            # [RC, P, KT, RCHUNK]

    return [{"xB": x_blocked(c), "wB": wB, "bT": bT} for c in range(N_CORES)]


def _gather(res):
    out = np.concatenate(
        [np.asarray(res.results[c]["outT"], dtype=np.float32).T
         for c in range(N_CORES)], axis=0)
    return np.ascontiguousarray(out)


def kernel(x, W, b):
    from concourse.bass_utils import run_bass_kernel_spmd

    if "nc" not in _cached:
        _cached["nc"] = _build()
    nc = _cached["nc"]

    in_maps = _make_in_maps(x, W, b)
    res = run_bass_kernel_spmd(nc, in_maps, list(range(N_CORES)))
    return _gather(res)


# revision 12
# speedup vs baseline: 1.0426x; 1.0426x over previous
"""Trainium2 Bass kernel for nn_Dense_BinaryLayer (binary-weight dense layer).

out = x @ Wb + b, where Wb = binarize(W) in {-1, +1}.

Strategy: data-parallel over the 8 NeuronCores — each core handles 2048 rows
of x with replicated W and b; no collectives.  Host-side prep is layout-only
byte movement: x is sharded and re-blocked so every DMA lands as 128
contiguous >=2KB descriptors (HWDGE descriptor generation is ~5ns/desc and
serial, so descriptor count — not just bytes — gates the pipe), W is passed
as its upper 2 bytes per f32 (== truncated bf16; only the sign/threshold
bit matters for binarize) halving W traffic, and b is passed as [128, 8] so
bias lands on partitions.

Per core (v3 — W-stationary, x-moving, transposed output):
  - One HWDGE queue (nc.sync) carries all bulk traffic in exact need-order:
    (W k-slab, x rc0 k-slab) pairs, then x rc1 halves, rc2, rc3, then
    stores.  No SWDGE, no cast DMAs (xT is declared float32r, bit-identical
    to f32 — the PE rounds internally).
  - W slab kt binarizes on DVE the moment it lands, one fused op:
    wb = (W > 2^-24) - 0.5 in {-0.5,+0.5} f32r; the missing x2 is folded
    into the eviction scale.
  - Matmul: stationary = wb block [128k x 128j], moving = x [128k x 512r],
    PSUM bank = [128j x 512r]; 8 banks = one 512-row block across all 1024
    output columns.  k-outer for row-blocks 0-1 (consumes W slabs in
    arrival order), j-outer for 2-3 (staggers bank completion so eviction
    overlaps the next block).
  - 6 dummy matmuls on a zeroed tile warm the PE HAM clock gate
    (1.2 -> 2.4 GHz) during the DMA ramp, ending just as real data lands.
  - Evictions alternate ACT/DVE: out_bf16 = 2*psum + b[j] (per-partition
    bias).  bf16 output halves store traffic; ~3e-3 max rel err (budget
    2e-2).  Host: out = concat(outT.T per core).astype(f32).
"""
import sys

sys.path.insert(0, "/opt/trn_rl_repo")

import numpy as np

N_TOTAL = 16384
D_IN = 1024
D_OUT = 1024
N_CORES = 8
ROWS = N_TOTAL // N_CORES      # 2048 rows per core
P = 128
KT = D_IN // P                 # 8 k-tiles
JT = D_OUT // P                # 8 j-tiles (psum banks)
RC = 4                         # row chunks per core
RCHUNK = ROWS // RC            # 512 rows = one full PSUM bank of f32
BIN_THRESH = 2.0 ** -24

_cached = {}


def _build():
    import concourse.tile as tile
    from concourse import bacc, mybir

    f32 = mybir.dt.float32
    f32r = mybir.dt.float32r
    bf16 = mybir.dt.bfloat16
    TS = mybir.AluOpType
    ACT = mybir.ActivationFunctionType

    nc = bacc.Bacc()
    # x, blocked per row-chunk: xB[rc, p, kt, r] = x[rc*512+r, kt*128+p]
    x_d = nc.declare_dram_parameter("xB", [RC, P, KT, RCHUNK], bf16,
                                    isOutput=False)
    # W as truncated bf16, blocked: wB[p, kt, j] = bf16(W[kt*128+p, j])
    w_d = nc.declare_dram_parameter("wB", [P, KT, D_OUT], bf16, isOutput=False)
    b_d = nc.declare_dram_parameter("bT", [P, JT], f32, isOutput=False)
    o_d = nc.declare_dram_parameter("outT", [D_OUT, ROWS], bf16, isOutput=True)

    with tile.TileContext(nc) as tc:
        with (
            tc.tile_pool(name="const", bufs=1) as const,
            tc.tile_pool(name="wpool", bufs=1) as wpool,
            tc.tile_pool(name="xp", bufs=4) as xp,
            tc.tile_pool(name="outp", bufs=32) as outp,
            tc.tile_pool(name="pso", bufs=8, space="PSUM") as pso,
        ):
            # --- persistent SBUF tiles
            warm = const.tile([P, RCHUNK], bf16, tag="warm")
            bias = const.tile([P, JT], f32, tag="bias")
            w_raw = wpool.tile([P, KT, D_OUT], bf16, tag="wraw")
            wb = wpool.tile([P, KT, D_OUT], bf16, tag="wb")
            x_t = [xp.tile([P, KT, RCHUNK], bf16, tag="x", name=f"x{rc}")
                   for rc in range(RC)]
            ps = [pso.tile([P, RCHUNK], f32, tag="ps", name=f"ps{j}")
                  for j in range(JT)]

            # --- HAM warm-up: PE busy from ~0.3us after engine start so the
            # clock gate flips to 8/8 as real data lands (bank 7 is reset by
            # the first real start=True matmul)
            nc.vector.memset(warm[:], 0.0)
            for _ in range(8):
                nc.tensor.matmul(ps[7][:], warm[:, 0:P], warm[:],
                                 start=True, stop=True)

            # --- bias on the second HWDGE queue (tiny, done early)
            nc.scalar.dma_start(bias[:], b_d[:])

            # --- sync HWDGE queue, strict need-order; binarize each W slab
            # on DVE the moment it lands
            for kt in range(KT):
                nc.sync.dma_start(w_raw[:, kt, :], w_d[:, kt, :])
                nc.sync.dma_start(x_t[0][:, kt, :], x_d[0, :, kt, :])
                nc.vector.tensor_scalar(
                    wb[:, kt, :], w_raw[:, kt, :], BIN_THRESH, 0.5,
                    TS.is_gt, TS.subtract,
                )
            nc.sync.dma_start(x_t[1][:, 0:4, :], x_d[1, :, 0:4, :])
            nc.sync.dma_start(x_t[1][:, 4:8, :], x_d[1, :, 4:8, :])
            nc.sync.dma_start(x_t[2][:], x_d[2])
            nc.sync.dma_start(x_t[3][:], x_d[3])

            def evict_store(rc, jt):
                # out_bf16 = 2*psum + b[j]; alternate engines so bank drain
                # keeps pace with the matmul stream
                o_sb = outp.tile([P, RCHUNK], bf16, tag="o", name=f"o{rc}_{jt}")
                bcol = bias[:, jt:jt + 1]
                if jt % 2 == 0:
                    nc.scalar.activation(o_sb[:], ps[jt][:], ACT.Identity,
                                         bias=bcol, scale=2.0)
                else:
                    nc.vector.tensor_scalar(o_sb[:], ps[jt][:], 2.0, bcol,
                                            TS.mult, TS.add)
                nc.sync.dma_start(
                    o_d[jt * P:(jt + 1) * P, rc * RCHUNK:(rc + 1) * RCHUNK],
                    o_sb[:])

            def mm(rc, jt, kt):
                nc.tensor.matmul(
                    ps[jt][:],
                    wb[:, kt, jt * P:(jt + 1) * P],
                    x_t[rc][:, kt, :],
                    start=(kt == 0), stop=(kt == KT - 1),
                )

            # row-blocks 0-1: k-outer (follows W/x arrival order)
            for rc in range(2):
                for kt in range(KT):
                    for jt in range(JT):
                        mm(rc, jt, kt)
                for jt in range(JT):
                    evict_store(rc, jt)
            # row-blocks 2-3: j-outer (staggered bank completion); rc3 ends
            # on an ACT-evicted group for a slightly shorter tail
            for rc, jts in ((2, range(JT)), (3, (0, 1, 2, 3, 4, 5, 7, 6))):
                for jt in jts:
                    for kt in range(KT):
                        mm(rc, jt, kt)
                    evict_store(rc, jt)

    nc.compile()
    nc.finalize()
    return nc


def _make_in_maps(x, W, b):
    import ml_dtypes

    x = np.asarray(x, dtype=np.float32)
    W = np.ascontiguousarray(np.asarray(W, dtype=np.float32))
    # upper 2 bytes of each little-endian f32 == truncated bf16
    w_hi = W.view(np.uint16).reshape(D_IN, D_OUT, 2)[:, :, 1]
    wB = np.ascontiguousarray(
        w_hi.reshape(KT, P, D_OUT).transpose(1, 0, 2)).view(ml_dtypes.bfloat16)
    bT = np.ascontiguousarray(
        np.asarray(b, dtype=np.float32).reshape(JT, P).T)

    # x as truncated bf16 (upper 2 bytes of each f32) — byte movement only
    x_hi = x.view(np.uint16).reshape(N_TOTAL, D_IN, 2)[:, :, 1]

    def x_blocked(c):
        xT = x_hi[c * ROWS:(c + 1) * ROWS].T       # [D_IN, ROWS] uint16
        xB = xT.reshape(KT, P, RC, RCHUNK).transpose(2, 1, 0, 3)
        return np.ascontiguousarray(xB).view(ml_dtypes.bfloat16)

    return [{"xB": x_blocked(c), "wB": wB, "bT": bT} for c in range(N_CORES)]


def _gather(res):
    out = np.concatenate(
        [np.asarray(res.results[c]["outT"], dtype=np.float32).T
         for c in range(N_CORES)], axis=0)
    return np.ascontiguousarray(out)


def kernel(x, W, b):
    from concourse.bass_utils import run_bass_kernel_spmd

    if "nc" not in _cached:
        _cached["nc"] = _build()
    nc = _cached["nc"]

    in_maps = _make_in_maps(x, W, b)
    res = run_bass_kernel_spmd(nc, in_maps, list(range(N_CORES)))
    return _gather(res)


# revision 14
# speedup vs baseline: 1.1604x; 1.1130x over previous
"""Trainium2 Bass kernel for nn_Dense_BinaryLayer (binary-weight dense layer).

out = x @ Wb + b, where Wb = binarize(W) in {-1, +1}.

Strategy: data-parallel over the 8 NeuronCores — each core handles 2048 rows
of x with replicated W and b; no collectives.  Host-side prep is layout-only
byte movement: x is sharded and re-blocked so every DMA lands as 128
contiguous >=2KB descriptors (HWDGE descriptor generation is ~5ns/desc and
serial, so descriptor count — not just bytes — gates the pipe), W is passed
as its upper 2 bytes per f32 (== truncated bf16; only the sign/threshold
bit matters for binarize) halving W traffic, and b is passed as [128, 8] so
bias lands on partitions.

Per core (v3 — W-stationary, x-moving, transposed output):
  - One HWDGE queue (nc.sync) carries all bulk traffic in exact need-order:
    (W k-slab, x rc0 k-slab) pairs, then x rc1 halves, rc2, rc3, then
    stores.  No SWDGE, no cast DMAs (xT is declared float32r, bit-identical
    to f32 — the PE rounds internally).
  - W slab kt binarizes on DVE the moment it lands, one fused op:
    wb = (W > 2^-24) - 0.5 in {-0.5,+0.5} f32r; the missing x2 is folded
    into the eviction scale.
  - Matmul: stationary = wb block [128k x 128j], moving = x [128k x 512r],
    PSUM bank = [128j x 512r]; 8 banks = one 512-row block across all 1024
    output columns.  k-outer for row-blocks 0-1 (consumes W slabs in
    arrival order), j-outer for 2-3 (staggers bank completion so eviction
    overlaps the next block).
  - 6 dummy matmuls on a zeroed tile warm the PE HAM clock gate
    (1.2 -> 2.4 GHz) during the DMA ramp, ending just as real data lands.
  - Evictions alternate ACT/DVE: out_bf16 = 2*psum + b[j] (per-partition
    bias).  bf16 output halves store traffic; ~3e-3 max rel err (budget
    2e-2).  Host: out = concat(outT.T per core).astype(f32).
"""
import sys

sys.path.insert(0, "/opt/trn_rl_repo")

import numpy as np

N_TOTAL = 16384
D_IN = 1024
D_OUT = 1024
N_CORES = 8
ROWS = N_TOTAL // N_CORES      # 2048 rows per core
P = 128
KT = D_IN // P                 # 8 k-tiles
JT = D_OUT // P                # 8 j-tiles (psum banks)
RC = 4                         # row chunks per core
RCHUNK = ROWS // RC            # 512 rows = one full PSUM bank of f32
BIN_THRESH = 2.0 ** -24

_cached = {}


def _build():
    import concourse.tile as tile
    from concourse import bacc, mybir

    f32 = mybir.dt.float32
    f32r = mybir.dt.float32r
    bf16 = mybir.dt.bfloat16
    TS = mybir.AluOpType
    ACT = mybir.ActivationFunctionType

    nc = bacc.Bacc()
    # x, blocked per row-chunk: xB[rc, p, kt, r] = x[rc*512+r, kt*128+p]
    x_d = nc.declare_dram_parameter("xB", [RC, P, KT, RCHUNK], f32r,
                                    isOutput=False)
    # W as truncated bf16, blocked: wB[p, kt, j] = bf16(W[kt*128+p, j])
    w_d = nc.declare_dram_parameter("wB", [P, KT, D_OUT], bf16, isOutput=False)
    b_d = nc.declare_dram_parameter("bT", [P, JT], f32, isOutput=False)
    o_d = nc.declare_dram_parameter("outT", [D_OUT, ROWS], bf16, isOutput=True)

    with tile.TileContext(nc) as tc:
        with (
            tc.tile_pool(name="const", bufs=1) as const,
            tc.tile_pool(name="wpool", bufs=1) as wpool,
            tc.tile_pool(name="xp", bufs=4) as xp,
            tc.tile_pool(name="outp", bufs=32) as outp,
            tc.tile_pool(name="pso", bufs=8, space="PSUM") as pso,
        ):
            # --- persistent SBUF tiles
            warm = const.tile([P, RCHUNK], f32, tag="warm")
            bias = const.tile([P, JT], f32, tag="bias")
            w_raw = wpool.tile([P, KT, D_OUT], bf16, tag="wraw")
            wb = wpool.tile([P, KT, D_OUT], f32r, tag="wb")
            x_t = [xp.tile([P, KT, RCHUNK], f32r, tag="x", name=f"x{rc}")
                   for rc in range(RC)]
            ps = [pso.tile([P, RCHUNK], f32, tag="ps", name=f"ps{j}")
                  for j in range(JT)]

            # --- HAM warm-up: PE busy from ~0.3us after engine start so the
            # clock gate flips to 8/8 as real data lands (bank 7 is reset by
            # the first real start=True matmul)
            nc.vector.memset(warm[:], 0.0)
            warm_r = warm[:].bitcast(f32r)
            for _ in range(7):
                nc.tensor.matmul(ps[7][:], warm_r[:, 0:P], warm_r[:],
                                 start=True, stop=True)

            # --- bias on the second HWDGE queue (tiny, done early)
            nc.scalar.dma_start(bias[:], b_d[:])

            # --- sync HWDGE queue, strict need-order; binarize each W slab
            # on DVE the moment it lands
            for kt in range(KT):
                nc.sync.dma_start(w_raw[:, kt, :], w_d[:, kt, :])
                nc.sync.dma_start(x_t[0][:, kt, :], x_d[0, :, kt, :])
                nc.vector.tensor_scalar(
                    wb[:, kt, :], w_raw[:, kt, :], BIN_THRESH, 0.5,
                    TS.is_gt, TS.subtract,
                )
            nc.sync.dma_start(x_t[1][:, 0:4, :], x_d[1, :, 0:4, :])
            nc.sync.dma_start(x_t[1][:, 4:8, :], x_d[1, :, 4:8, :])
            nc.sync.dma_start(x_t[2][:], x_d[2])
            nc.sync.dma_start(x_t[3][:], x_d[3])

            def evict_store(rc, jt):
                # out_bf16 = 2*psum + b[j]; alternate engines so bank drain
                # keeps pace with the matmul stream
                o_sb = outp.tile([P, RCHUNK], bf16, tag="o", name=f"o{rc}_{jt}")
                bcol = bias[:, jt:jt + 1]
                if jt % 2 == 0:
                    nc.scalar.activation(o_sb[:], ps[jt][:], ACT.Identity,
                                         bias=bcol, scale=2.0)
                else:
                    nc.vector.tensor_scalar(o_sb[:], ps[jt][:], 2.0, bcol,
                                            TS.mult, TS.add)
                nc.sync.dma_start(
                    o_d[jt * P:(jt + 1) * P, rc * RCHUNK:(rc + 1) * RCHUNK],
                    o_sb[:])

            def mm(rc, jt, kt):
                nc.tensor.matmul(
                    ps[jt][:],
                    wb[:, kt, jt * P:(jt + 1) * P],
                    x_t[rc][:, kt, :],
                    start=(kt == 0), stop=(kt == KT - 1),
                )

            # row-blocks 0-1: k-outer (follows W/x arrival order)
            for rc in range(2):
                for kt in range(KT):
                    for jt in range(JT):
                        mm(rc, jt, kt)
                for jt in range(JT):
                    evict_store(rc, jt)
            # row-blocks 2-3: j-outer (staggered bank completion); rc3 ends
            # on an ACT-evicted group for a slightly shorter tail
            for rc, jts in ((2, range(JT)), (3, (0, 1, 2, 3, 4, 5, 7, 6))):
                for jt in jts:
                    for kt in range(KT):
                        mm(rc, jt, kt)
                    evict_store(rc, jt)

    nc.compile()
    nc.finalize()
    return nc


def _make_in_maps(x, W, b):
    import ml_dtypes

    x = np.asarray(x, dtype=np.float32)
    W = np.ascontiguousarray(np.asarray(W, dtype=np.float32))
    # upper 2 bytes of each little-endian f32 == truncated bf16
    w_hi = W.view(np.uint16).reshape(D_IN, D_OUT, 2)[:, :, 1]
    wB = np.ascontiguousarray(
        w_hi.reshape(KT, P, D_OUT).transpose(1, 0, 2)).view(ml_dtypes.bfloat16)
    bT = np.ascontiguousarray(
        np.asarray(b, dtype=np.float32).reshape(JT, P).T)

    def x_blocked(c):
        xT = x[c * ROWS:(c + 1) * ROWS].T          # [D_IN, ROWS]
        xB = xT.reshape(KT, P, RC, RCHUNK).transpose(2, 1, 0, 3)
        return np.ascontiguousarray(xB)            # [RC, P, KT, RCHUNK]

    return [{"xB": x_blocked(c), "wB": wB, "bT": bT} for c in range(N_CORES)]


def _gather(res):
    out = np.concatenate(
        [np.asarray(res.results[c]["outT"], dtype=np.float32).T
         for c in range(N_CORES)], axis=0)
    return np.ascontiguousarray(out)


def kernel(x, W, b):
    from concourse.bass_utils import run_bass_kernel_spmd

    if "nc" not in _cached:
        _cached["nc"] = _build()
    nc = _cached["nc"]

    in_maps = _make_in_maps(x, W, b)
    res = run_bass_kernel_spmd(nc, in_maps, list(range(N_CORES)))
    return _gather(res)


# revision 18
# speedup vs baseline: 1.1642x; 1.0033x over previous
"""Trainium2 Bass kernel for nn_Dense_BinaryLayer (binary-weight dense layer).

out = x @ Wb + b, where Wb = binarize(W) in {-1, +1}.

Strategy: data-parallel over the 8 NeuronCores — each core handles 2048 rows
of x with replicated W and b; no collectives.  Host-side prep is layout-only
byte movement: x is sharded and re-blocked so every DMA lands as 128
contiguous >=2KB descriptors (HWDGE descriptor generation is ~5ns/desc and
serial, so descriptor count — not just bytes — gates the pipe), W is passed
as its upper 2 bytes per f32 (== truncated bf16; only the sign/threshold
bit matters for binarize) halving W traffic, and b is passed as [128, 8] so
bias lands on partitions.

Per core (v3 — W-stationary, x-moving, transposed output):
  - One HWDGE queue (nc.sync) carries all bulk traffic in exact need-order:
    (W k-slab, x rc0 k-slab) pairs, then x rc1 halves, rc2, rc3, then
    stores.  No SWDGE, no cast DMAs (xT is declared float32r, bit-identical
    to f32 — the PE rounds internally).
  - W slab kt binarizes on DVE the moment it lands, one fused op:
    wb = (W > 2^-24) - 0.5 in {-0.5,+0.5} f32r; the missing x2 is folded
    into the eviction scale.
  - Matmul: stationary = wb block [128k x 128j], moving = x [128k x 512r],
    PSUM bank = [128j x 512r]; 8 banks = one 512-row block across all 1024
    output columns.  k-outer for row-blocks 0-1 (consumes W slabs in
    arrival order), j-outer for 2-3 (staggers bank completion so eviction
    overlaps the next block).
  - 7 dummy matmuls on a zeroed tile warm the PE HAM clock gate
    (1.2 -> 2.4 GHz) during the DMA ramp, ending just as real data lands.
  - Evictions alternate ACT/DVE: out_bf16 = 2*psum + b[j] (per-partition
    bias).  bf16 output halves store traffic; ~3e-3 max rel err (budget
    2e-2).  Host: out = concat(outT.T per core).astype(f32).
"""
import sys

sys.path.insert(0, "/opt/trn_rl_repo")

import numpy as np

N_TOTAL = 16384
D_IN = 1024
D_OUT = 1024
N_CORES = 8
ROWS = N_TOTAL // N_CORES      # 2048 rows per core
P = 128
KT = D_IN // P                 # 8 k-tiles
JT = D_OUT // P                # 8 j-tiles (psum banks)
RC = 4                         # row chunks per core
RCHUNK = ROWS // RC            # 512 rows = one full PSUM bank of f32
BIN_THRESH = 2.0 ** -24

_cached = {}


def _build():
    import concourse.tile as tile
    from concourse import bacc, mybir

    f32 = mybir.dt.float32
    f32r = mybir.dt.float32r
    bf16 = mybir.dt.bfloat16
    TS = mybir.AluOpType
    ACT = mybir.ActivationFunctionType

    nc = bacc.Bacc()
    # x, blocked per row-chunk: xB[rc, p, kt, r] = x[rc*512+r, kt*128+p]
    x_d = nc.declare_dram_parameter("xB", [RC, P, KT, RCHUNK], f32r,
                                    isOutput=False)
    # W as truncated bf16, blocked: wB[p, kt, j] = bf16(W[kt*128+p, j])
    w_d = nc.declare_dram_parameter("wB", [P, KT, D_OUT], bf16, isOutput=False)
    b_d = nc.declare_dram_parameter("bT", [P, JT], f32, isOutput=False)
    o_d = nc.declare_dram_parameter("outT", [D_OUT, ROWS], bf16, isOutput=True)

    with tile.TileContext(nc) as tc:
        with (
            tc.tile_pool(name="const", bufs=1) as const,
            tc.tile_pool(name="wpool", bufs=1) as wpool,
            tc.tile_pool(name="xp", bufs=4) as xp,
            tc.tile_pool(name="outp", bufs=32) as outp,
            tc.tile_pool(name="pso", bufs=8, space="PSUM") as pso,
        ):
            # --- persistent SBUF tiles
            warm = const.tile([P, RCHUNK], f32, tag="warm")
            bias = const.tile([P, JT], f32, tag="bias")
            w_raw = wpool.tile([P, KT, D_OUT], bf16, tag="wraw")
            wb = wpool.tile([P, KT, D_OUT], f32r, tag="wb")
            x_t = [xp.tile([P, KT, RCHUNK], f32r, tag="x", name=f"x{rc}")
                   for rc in range(RC)]
            ps = [pso.tile([P, RCHUNK], f32, tag="ps", name=f"ps{j}")
                  for j in range(JT)]

            # --- HAM warm-up: PE busy from ~0.3us after engine start so the
            # clock gate flips to 8/8 as real data lands (bank 7 is reset by
            # the first real start=True matmul)
            nc.vector.memset(warm[:], 0.0)
            warm_r = warm[:].bitcast(f32r)
            for _ in range(7):
                nc.tensor.matmul(ps[7][:], warm_r[:, 0:P], warm_r[:],
                                 start=True, stop=True)

            # --- bias on the second HWDGE queue (tiny, done early)
            nc.scalar.dma_start(bias[:], b_d[:])

            # --- sync HWDGE queue, strict need-order; binarize each W slab
            # on DVE the moment it lands.  kt0 is split in j-halves (with
            # x00 between) so the first matmuls start one bin-op earlier.
            H = D_OUT // 2
            nc.sync.dma_start(w_raw[:, 0, 0:H], w_d[:, 0, 0:H])
            nc.vector.tensor_scalar(
                wb[:, 0, 0:H], w_raw[:, 0, 0:H], BIN_THRESH, 0.5,
                TS.is_gt, TS.subtract,
            )
            nc.sync.dma_start(x_t[0][:, 0, :], x_d[0, :, 0, :])
            nc.sync.dma_start(w_raw[:, 0, H:D_OUT], w_d[:, 0, H:D_OUT])
            nc.vector.tensor_scalar(
                wb[:, 0, H:D_OUT], w_raw[:, 0, H:D_OUT], BIN_THRESH, 0.5,
                TS.is_gt, TS.subtract,
            )
            for kt in range(1, KT):
                nc.sync.dma_start(w_raw[:, kt, :], w_d[:, kt, :])
                nc.sync.dma_start(x_t[0][:, kt, :], x_d[0, :, kt, :])
                nc.vector.tensor_scalar(
                    wb[:, kt, :], w_raw[:, kt, :], BIN_THRESH, 0.5,
                    TS.is_gt, TS.subtract,
                )
            nc.sync.dma_start(x_t[1][:, 0:4, :], x_d[1, :, 0:4, :])
            nc.sync.dma_start(x_t[1][:, 4:8, :], x_d[1, :, 4:8, :])
            nc.sync.dma_start(x_t[2][:], x_d[2])
            nc.sync.dma_start(x_t[3][:], x_d[3])

            def evict_store(rc, jt, last=False):
                # out_bf16 = 2*psum + b[j]; alternate engines so bank drain
                # keeps pace with the matmul stream.  The final store rides
                # the scalar engine's own HWDGE queue right after its ACT
                # eviction — no cross-engine semaphore hop in the tail.
                o_sb = outp.tile([P, RCHUNK], bf16, tag="o", name=f"o{rc}_{jt}")
                bcol = bias[:, jt:jt + 1]
                if jt % 2 == 0:
                    nc.scalar.activation(o_sb[:], ps[jt][:], ACT.Identity,
                                         bias=bcol, scale=2.0)
                else:
                    nc.vector.tensor_scalar(o_sb[:], ps[jt][:], 2.0, bcol,
                                            TS.mult, TS.add)
                dma_eng = nc.scalar if last else nc.sync
                dma_eng.dma_start(
                    o_d[jt * P:(jt + 1) * P, rc * RCHUNK:(rc + 1) * RCHUNK],
                    o_sb[:])

            def mm(rc, jt, kt):
                nc.tensor.matmul(
                    ps[jt][:],
                    wb[:, kt, jt * P:(jt + 1) * P],
                    x_t[rc][:, kt, :],
                    start=(kt == 0), stop=(kt == KT - 1),
                )

            # row-blocks 0-1: k-outer (follows W/x arrival order)
            for rc in range(2):
                for kt in range(KT):
                    for jt in range(JT):
                        mm(rc, jt, kt)
                for jt in range(JT):
                    evict_store(rc, jt)
            # row-blocks 2-3: j-outer (staggered bank completion); rc3 ends
            # on an ACT-evicted group for a slightly shorter tail
            for rc, jts in ((2, range(JT)), (3, (0, 1, 2, 3, 4, 5, 7, 6))):
                for jt in jts:
                    for kt in range(KT):
                        mm(rc, jt, kt)
                    evict_store(rc, jt, last=(rc == 3 and jt == 6))

    nc.compile()
    nc.finalize()
    return nc


def _make_in_maps(x, W, b):
    import ml_dtypes

    x = np.asarray(x, dtype=np.float32)
    W = np.ascontiguousarray(np.asarray(W, dtype=np.float32))
    # upper 2 bytes of each little-endian f32 == truncated bf16
    w_hi = W.view(np.uint16).reshape(D_IN, D_OUT, 2)[:, :, 1]
    wB = np.ascontiguousarray(
        w_hi.reshape(KT, P, D_OUT).transpose(1, 0, 2)).view(ml_dtypes.bfloat16)
    bT = np.ascontiguousarray(
        np.asarray(b, dtype=np.float32).reshape(JT, P).T)

    def x_blocked(c):
        xT = x[c * ROWS:(c + 1) * ROWS].T          # [D_IN, ROWS]
        xB = xT.reshape(KT, P, RC, RCHUNK).transpose(2, 1, 0, 3)
        return np.ascontiguousarray(xB)            # [RC, P, KT, RCHUNK]

    return [{"xB": x_blocked(c), "wB": wB, "bT": bT} for c in range(N_CORES)]


def _gather(res):
    out = np.concatenate(
        [np.asarray(res.results[c]["outT"], dtype=np.float32).T
         for c in range(N_CORES)], axis=0)
    return np.ascontiguousarray(out)


def kernel(x, W, b):
    from concourse.bass_utils import run_bass_kernel_spmd

    if "nc" not in _cached:
        _cached["nc"] = _build()
    nc = _cached["nc"]

    in_maps = _make_in_maps(x, W, b)
    res = run_bass_kernel_spmd(nc, in_maps, list(range(N_CORES)))
    return _gather(res)
